# revision 1
# baseline (speedup 1.0000x reference)
"""Mamba-1 SSM block (LayerNorm -> in_proj -> causal conv -> selective scan
-> gated out_proj -> relu + residual) on 8 Trainium2 NeuronCores.

Sharding: core c handles batch b = c//2 and d_inner half h = c%2.
Each core computes the x-path (in_proj x part, conv, x_dbl) for ALL
d_inner channels (cheap duplication that avoids a mid-pipeline collective),
but runs delta/scan/gating only for its own 768 channels.  out_proj is
computed over own channels for all 1024 timesteps; a pair AllReduce sums
the two channel-half partials, after which both cores of a pair hold the
full output (the host keeps the even core's copy).

Channel order is permuted per-core to [own 768, peer 768] on the host so
the SPMD program can index "own half" statically.
"""

import numpy as np
import ml_dtypes
from contextlib import ExitStack

import concourse.bass as bass
import concourse.bacc as bacc
import concourse.tile as tile
from concourse import mybir
from concourse.bass_utils import run_bass_kernel_spmd
from concourse.masks import make_identity

F32 = mybir.dt.float32
BF16 = mybir.dt.bfloat16
NPBF16 = ml_dtypes.bfloat16
OP = mybir.AluOpType
AF = mybir.ActivationFunctionType

B, L, D = 4, 1024, 768
DI = 1536          # d_inner
DH = 768           # d_inner half per core
N = 16             # d_state
DCONV = 4
DTR = 48           # dt_rank
EPS = 1e-6
EPROJ = DI + DH    # in_proj output channels per core: full x + own z
TC = 128           # scan time-chunk
CFG = {"hc": "gpsimd", "du": "vector", "r8": "vector", "evict": "scalar", "scan_bufs": 6, "bc_bufs": 2, "stop": "full", "part_bf16": True, "da": "gpsimd", "dbu": "gpsimd", "carry": "vector", "ms": "vector", "xpad_bufs": 4, "lnp_bufs": 3}
NCH = L // TC


def ap_view(t, extra_off, ap_list):
    """Build a custom AP over an existing tile's storage."""
    return bass.AP(tensor=t.tensor, offset=t.offset + extra_off, ap=ap_list)


def nc_finish(nc):
    return nc


def build_program():
    nc = bacc.Bacc(num_devices=8)

    x_in = nc.dram_tensor("x_in", [L, D], F32, kind="ExternalInput")
    w_in_t = nc.dram_tensor("w_in_t", [D, EPROJ], BF16, kind="ExternalInput")
    bias_in = nc.dram_tensor("bias_in", [EPROJ], F32, kind="ExternalInput")
    w_conv = nc.dram_tensor("w_conv", [DI, DCONV], F32, kind="ExternalInput")
    b_conv = nc.dram_tensor("b_conv", [DI], F32, kind="ExternalInput")
    w_x_t = nc.dram_tensor("w_x_t", [DI, DTR + 2 * N], BF16, kind="ExternalInput")
    w_dt_t = nc.dram_tensor("w_dt_t", [DTR, DH], BF16, kind="ExternalInput")
    b_dt = nc.dram_tensor("b_dt", [DH], F32, kind="ExternalInput")
    a_neg = nc.dram_tensor("a_neg", [DH, N], BF16, kind="ExternalInput")
    d_par = nc.dram_tensor("d_par", [DH], F32, kind="ExternalInput")
    w_out_t = nc.dram_tensor("w_out_t", [DH, D], BF16, kind="ExternalInput")
    out_d = nc.dram_tensor("out", [L, D], F32, kind="ExternalOutput")

    bc_bounce = nc.dram_tensor("bc_bounce", [2 * N, L], BF16)
    PDT = BF16 if CFG["part_bf16"] else F32
    part_ds = [nc.dram_tensor(f"part_d{i}", [L // 2, D], PDT) for i in range(2)]
    sum_ds = [nc.dram_tensor(f"sum_d{i}", [L // 2, D], PDT) for i in range(2)]

    with tile.TileContext(nc) as tc, ExitStack() as ctx:
        consts = ctx.enter_context(tc.tile_pool(name="consts", bufs=1))
        wpool = ctx.enter_context(tc.tile_pool(name="wpool", bufs=1))
        lnp = ctx.enter_context(tc.tile_pool(name="lnp", bufs=3))
        xnt_p = ctx.enter_context(tc.tile_pool(name="xnt", bufs=1))
        actp = ctx.enter_context(tc.tile_pool(name="actp", bufs=1))
        rot = ctx.enter_context(tc.tile_pool(name="rot", bufs=4))
        scanp = ctx.enter_context(tc.tile_pool(name="scanp", bufs=2))
        outp = ctx.enter_context(tc.tile_pool(name="outp", bufs=2))
        psum = ctx.enter_context(tc.tile_pool(name="psum", bufs=6, space="PSUM"))
        pst = ctx.enter_context(tc.tile_pool(name="pst", bufs=2, space="PSUM"))

        # ---------------- constants ----------------
        ident = consts.tile([128, 128], BF16)
        make_identity(nc, ident)
        eps_t = consts.tile([128, 1], F32)
        nc.vector.memset(eps_t, EPS)
        wconv_t = consts.tile([128, 12, DCONV], F32)
        nc.sync.dma_start(out=wconv_t, in_=w_conv[:].rearrange("(g p) k -> p g k", p=128))
        bconv_t = consts.tile([128, 12], F32)
        nc.sync.dma_start(out=bconv_t, in_=b_conv[:].rearrange("(g p) -> p g", p=128))
        bdt_t = consts.tile([128, 6], F32)
        nc.sync.dma_start(out=bdt_t, in_=b_dt[:].rearrange("(g p) -> p g", p=128))
        dpar_t = consts.tile([128, 6], F32)
        nc.sync.dma_start(out=dpar_t, in_=d_par[:].rearrange("(g p) -> p g", p=128))
        a_t = consts.tile([128, 6, N], BF16)
        nc.sync.dma_start(out=a_t, in_=a_neg[:].rearrange("(g p) n -> p g n", p=128))
        biasin_t = consts.tile([128, 18], F32)
        nc.sync.dma_start(out=biasin_t, in_=bias_in[:].rearrange("(m p) -> p m", p=128))

        # ---------------- weights ----------------
        w_in_sb = [wpool.tile([128, EPROJ], BF16, tag="w_in", bufs=CFG["scan_bufs"], name=f"w_in{k}") for k in range(6)]
        for k in range(6):
            nc.sync.dma_start(out=w_in_sb[k], in_=w_in_t[k * 128:(k + 1) * 128, :])
        w_x_sb = [wpool.tile([128, DTR + 2 * N], BF16, tag="w_x", bufs=12, name=f"w_x{k}") for k in range(12)]
        for k in range(12):
            nc.sync.dma_start(out=w_x_sb[k], in_=w_x_t[k * 128:(k + 1) * 128, :])
        w_dt_sb = wpool.tile([DTR, DH], BF16, tag="w_dt", bufs=1)
        nc.sync.dma_start(out=w_dt_sb, in_=w_dt_t[:])
        w_out_sb = [wpool.tile([128, D], BF16, tag="w_out", bufs=6, name=f"w_out{k}") for k in range(6)]
        for k in range(6):
            nc.sync.dma_start(out=w_out_sb[k], in_=w_out_t[k * 128:(k + 1) * 128, :])

        # ---------------- LayerNorm (z-score only; gamma/beta folded into W/bias)
        # x [t, d] tiles -> xn bf16 -> PE transpose -> xn_T [d, t]
        xn_T = [xnt_p.tile([128, L], BF16, tag="xn_T", bufs=6, name=f"xn_T{k}") for k in range(6)]
        x_all = actp.tile([128, 8, D], F32, tag="x_all", bufs=1)
        nc.sync.dma_start(out=x_all, in_=x_in[:].rearrange("(a p) d -> p a d", p=128))
        for tt in range(8):
            xt = x_all[:, tt, :]
            stats = lnp.tile([128, 3, 6], F32, tag="stats")
            for s in range(3):
                nc.vector.bn_stats(out=stats[:, s, :], in_=xt[:, s * 256:(s + 1) * 256])
            mv = lnp.tile([128, 2], F32, tag="mv")
            nc.vector.bn_aggr(out=mv, in_=stats)
            sd = lnp.tile([128, 1], F32, tag="sd")
            nc.scalar.activation(out=sd, in_=mv[:, 1:2], func=AF.Sqrt, bias=eps_t)
            rs = lnp.tile([128, 1], F32, tag="rs")
            nc.vector.reciprocal(out=rs, in_=sd)
            xnb = lnp.tile([128, D], BF16, tag="xnb", bufs=2)
            nc.vector.tensor_scalar(
                out=xnb, in0=xt, scalar1=mv[:, 0:1], scalar2=rs,
                op0=OP.subtract, op1=OP.mult)
            for dd in range(6):
                ps = pst.tile([128, 128], BF16, tag="ps_t")
                nc.tensor.transpose(ps, xnb[:, dd * 128:(dd + 1) * 128], ident)
                if CFG["evict"] == "vector":
                    nc.vector.tensor_copy(
                        out=xn_T[dd][:, tt * 128:(tt + 1) * 128], in_=ps)
                else:
                    nc.scalar.copy(
                        out=xn_T[dd][:, tt * 128:(tt + 1) * 128], in_=ps)

        # ---------------- in_proj: out[e, t] = W^T(kxm) @ xn_T(kxn) ----------
        # m-tiles 0..11 -> x (all DI, local order [own, peer]); 12..17 -> z own
        x_pad = [rot.tile([128, L + 3], BF16, tag="x_pad", bufs=CFG["xpad_bufs"], name=f"x_pad{k}") for k in range(12)]
        xc = [actp.tile([128, L], BF16, tag=("xc" if k < 6 else "xcp"), bufs=(6 if k < 6 else 3), name=f"xcg{k}") for k in range(12)]
        sz = [actp.tile([128, L], BF16, tag="sz", bufs=6, name=f"szg{k}") for k in range(6)]
        for m in range(18):
            for ns in range(2):
                ps = psum.tile([128, 512], F32, tag="ps_mm")
                for k in range(6):
                    nc.tensor.matmul(
                        ps, w_in_sb[k][:, m * 128:(m + 1) * 128],
                        xn_T[k][:, ns * 512:(ns + 1) * 512],
                        start=(k == 0), stop=(k == 5))
                if m < 12:
                    nc.scalar.activation(
                        out=x_pad[m][:, 3 + ns * 512: 3 + (ns + 1) * 512], in_=ps,
                        func=AF.Identity, bias=biasin_t[:, m:m + 1])
                else:
                    nc.scalar.activation(
                        out=sz[m - 12][:, ns * 512:(ns + 1) * 512], in_=ps,
                        func=AF.Silu, bias=biasin_t[:, m:m + 1])

        # ---------------- causal depthwise conv + silu -> xc ----------------
        for g in range(12):
            nc.vector.memset(x_pad[g][:, 0:3], 0.0)
            acc = rot.tile([128, L], BF16, tag="conv_acc", bufs=2)
            nc.vector.tensor_scalar_mul(acc, x_pad[g][:, 0:L], wconv_t[:, g, 0:1])
            for k in range(1, 4):
                nc.vector.scalar_tensor_tensor(
                    out=acc, in0=x_pad[g][:, k:k + L], scalar=wconv_t[:, g, k:k + 1],
                    in1=acc, op0=OP.mult, op1=OP.add)
            nc.scalar.activation(
                out=xc[g], in_=acc, func=AF.Silu, bias=bconv_t[:, g:g + 1])

        if CFG["stop"] == "conv":
            return nc_finish(nc)
        # ---------------- x_dbl = W_x^T @ xc  -> [80, t] ----------------
        dt_t = actp.tile([DTR, L], BF16, tag="dt_t", bufs=1)
        bc_sb = actp.tile([2 * N, L], BF16, tag="bc_sb", bufs=1)
        for ns in range(2):
            ps = psum.tile([128, 512], F32, tag="ps_mm")
            for k in range(12):
                nc.tensor.matmul(
                    ps[0:DTR + 2 * N, :], w_x_sb[k],
                    xc[k][:, ns * 512:(ns + 1) * 512],
                    start=(k == 0), stop=(k == 11))
            nc.vector.tensor_copy(
                out=bc_sb[:, ns * 512:(ns + 1) * 512], in_=ps[0:2 * N, :])
            nc.scalar.copy(
                out=dt_t[0:32, ns * 512:(ns + 1) * 512], in_=ps[32:64, :])
            nc.scalar.copy(
                out=dt_t[32:DTR, ns * 512:(ns + 1) * 512], in_=ps[64:2 * N + DTR, :])
        nc.sync.dma_start(out=bc_bounce[:], in_=bc_sb)

        if CFG["stop"] == "xdbl":
            return nc_finish(nc)
        # ---------------- delta = softplus(W_dt^T @ dt + b_dt) [d_own, t] ----
        delta = [actp.tile([128, L], BF16, tag="delta", bufs=6, name=f"deltag{k}") for k in range(6)]
        for m in range(6):
            for ns in range(2):
                ps = psum.tile([128, 512], F32, tag="ps_mm")
                nc.tensor.matmul(
                    ps, w_dt_sb[:, m * 128:(m + 1) * 128],
                    dt_t[:, ns * 512:(ns + 1) * 512], start=True, stop=True)
                spt = rot.tile([128, 512], F32, tag="spt", bufs=1)
                nc.scalar.activation(out=spt, in_=ps, func=AF.Exp,
                                     bias=bdt_t[:, m:m + 1])
                nc.scalar.activation(
                    out=delta[m][:, ns * 512:(ns + 1) * 512], in_=spt,
                    func=AF.Ln, bias=1.0)

        if CFG["stop"] == "delta":
            return nc_finish(nc)
        # ---------------- selective scan ----------------
        # layout: [128 chan, (n, t)] free; per (chunk, group):
        #   dA = exp(A_n * delta), dBu = delta*u*B; col 0 of each n-segment
        #   carries the running state between chunks (dA=0 there).
        ys = [actp.tile([128, L], BF16, tag="ys", bufs=6, name=f"ysg{k}") for k in range(6)]
        carry = [actp.tile([128, N], BF16, tag="carry", bufs=6, name=f"carryg{k}") for k in range(6)]
        for c in range(NCH):
            bbc = scanp.tile([128, N, TC], BF16, tag="bbc", bufs=CFG["bc_bufs"])
            nc.sync.dma_start(
                out=bbc,
                in_=ap_view(bc_bounce[:], c * TC, [[0, 128], [L, N], [1, TC]]))
            cbc = scanp.tile([128, N, TC], BF16, tag="cbc", bufs=CFG["bc_bufs"])
            nc.sync.dma_start(
                out=cbc,
                in_=ap_view(bc_bounce[:], N * L + c * TC, [[0, 128], [L, N], [1, TC]]))
            for g in range(6):
                dstep = delta[g].ap[0][0]
                ustep = xc[g].ap[0][0]
                # du = delta * u
                du = rot.tile([128, TC], BF16, tag="du", bufs=2)
                du_eng = nc.gpsimd if CFG["du"] == "gpsimd" else nc.vector
                du_eng.tensor_mul(
                    du, delta[g][:, c * TC:(c + 1) * TC], xc[g][:, c * TC:(c + 1) * TC])
                # dBu[:, n, 1:] = du (bcast n) * B
                dbu = wpool.tile([128, N, TC + 1], BF16, tag="w_in", bufs=CFG["scan_bufs"], name="dbu")
                dbu_eng = {"vector": nc.vector, "gpsimd": nc.gpsimd,
                           "split": (nc.gpsimd if c % 2 else nc.vector)}[CFG["dbu"]]
                dbu_eng.tensor_mul(
                    dbu[:, :, 1:],
                    ap_view(du, 0, [[du.ap[0][0], 128], [0, N], [1, TC]]),
                    bbc)
                ceng = nc.gpsimd if CFG["carry"] == "gpsimd" else nc.vector
                mseng = nc.gpsimd if CFG["ms"] == "gpsimd" else nc.vector
                if c == 0:
                    mseng.memset(dbu[:, :, 0:1], 0.0)
                else:
                    ceng.tensor_copy(
                        out=dbu[:, :, 0:1],
                        in_=ap_view(carry[g], 0, [[carry[g].ap[0][0], 128], [1, N], [0, 1]]))
                # dA[:, n, 1:] = exp(A_n * delta)
                da = wpool.tile([128, N, TC + 1], BF16, tag="w_in", bufs=CFG["scan_bufs"], name="da")
                da_eng = {"vector": nc.vector, "gpsimd": nc.gpsimd,
                          "split": (nc.gpsimd if c % 2 else nc.vector)}[CFG["da"]]
                da_eng.tensor_mul(
                    da[:, :, 1:],
                    ap_view(delta[g], c * TC, [[dstep, 128], [0, N], [1, TC]]),
                    ap_view(a_t, g * N, [[a_t.ap[0][0], 128], [1, N], [0, TC]]))
                nc.scalar.activation(out=da[:, :, 1:], in_=da[:, :, 1:], func=AF.Exp)
                mseng.memset(da[:, :, 0:1], 0.0)
                # scan along (n, t)
                h = wpool.tile([128, N, TC + 1], BF16, tag="w_in", bufs=CFG["scan_bufs"], name="h")
                nc.vector.tensor_tensor_scan(
                    out=h.rearrange("p a b -> p (a b)"),
                    data0=da.rearrange("p a b -> p (a b)"),
                    data1=dbu.rearrange("p a b -> p (a b)"),
                    initial=0.0, op0=OP.mult, op1=OP.add)
                ceng.tensor_copy(
                    out=carry[g],
                    in_=ap_view(h, TC, [[h.ap[0][0], 128], [TC + 1, N]]))
                # readout: hC -> (reuse dbu), tree-reduce over n -> ys
                hc_eng = {"split": (nc.gpsimd if g % 2 else nc.vector),
                          "vector": nc.vector, "gpsimd": nc.gpsimd}[CFG["hc"]]
                hc_eng.tensor_mul(dbu[:, :, 1:], h[:, :, 1:], cbc)
                r8_eng = nc.gpsimd if CFG["r8"] == "gpsimd" else nc.vector
                r8_eng.tensor_add(da[:, 0:8, 0:TC], dbu[:, 0:8, 1:], dbu[:, 8:16, 1:])
                nc.vector.tensor_add(da[:, 8:12, 0:TC], da[:, 0:4, 0:TC], da[:, 4:8, 0:TC])
                nc.vector.tensor_add(da[:, 12:14, 0:TC], da[:, 8:10, 0:TC], da[:, 10:12, 0:TC])
                ys_sl = ys[g][:, c * TC:(c + 1) * TC]
                nc.vector.tensor_add(ys_sl, da[:, 12, 0:TC], da[:, 13, 0:TC])
                nc.vector.scalar_tensor_tensor(
                    out=ys_sl, in0=xc[g][:, c * TC:(c + 1) * TC],
                    scalar=dpar_t[:, g:g + 1], in1=ys_sl, op0=OP.mult, op1=OP.add)
                nc.vector.tensor_mul(ys_sl, ys_sl, sz[g][:, c * TC:(c + 1) * TC])
            # out_proj m-tile for this chunk's time columns (m == c for TC=128)
            m = c
            part = outp.tile([128, D], PDT, tag="part")
            for ns, nw in ((0, 512), (1, 256)):
                ps = psum.tile([128, 512], F32, tag="ps_mm")
                for k in range(6):
                    nc.tensor.matmul(
                        ps[:, 0:nw], ys[k][:, m * 128:(m + 1) * 128],
                        w_out_sb[k][:, ns * 512:ns * 512 + nw],
                        start=(k == 0), stop=(k == 5))
                nc.vector.tensor_copy(
                    out=part[:, ns * 512:ns * 512 + nw], in_=ps[:, 0:nw])
            nc.sync.dma_start(
                out=part_ds[m // 4][(m % 4) * 128:(m % 4 + 1) * 128, :], in_=part)
            if m % 4 == 3:
                nc.gpsimd.collective_compute(
                    "AllReduce", OP.add,
                    replica_groups=[[0, 1], [2, 3], [4, 5], [6, 7]],
                    ins=[part_ds[m // 4][:]],
                    outs=[sum_ds[m // 4][:]])

        if CFG["stop"] in ("scan", "outproj"):
            return nc_finish(nc)

        # ---------------- relu + residual + store ----------------
        for m in range(8):
            s_sb = outp.tile([128, D], PDT, tag="s_sb")
            nc.sync.dma_start(
                out=s_sb, in_=sum_ds[m // 4][(m % 4) * 128:(m % 4 + 1) * 128, :])
            o2 = outp.tile([128, D], F32, tag="o2", bufs=2)
            nc.vector.tensor_scalar_max(o2, s_sb, 0.0)
            nc.vector.tensor_add(o2, o2, x_all[:, m, :])
            nc.sync.dma_start(out=out_d[m * 128:(m + 1) * 128, :], in_=o2)

    nc.compile()
    return nc


def make_in_maps(inputs):
    """Host-side sharding: per-core input dicts."""
    x = np.asarray(inputs["input_data"], np.float32)
    ln_g = np.asarray(inputs["ln_g"], np.float32)
    ln_b = np.asarray(inputs["ln_b"], np.float32)
    W_in = np.asarray(inputs["W_in"], np.float32)
    W_conv = np.asarray(inputs["W_conv"], np.float32)
    b_conv = np.asarray(inputs["b_conv"], np.float32)
    W_x = np.asarray(inputs["W_x"], np.float32)
    W_dt = np.asarray(inputs["W_dt"], np.float32)
    b_dt = np.asarray(inputs["b_dt"], np.float32)
    A_log = np.asarray(inputs["A_log"], np.float32)
    D_param = np.asarray(inputs["D_param"], np.float32)
    W_out = np.asarray(inputs["W_out"], np.float32)

    maps = []
    for c in range(8):
        b, h = c // 2, c % 2
        own = np.arange(h * DH, (h + 1) * DH)
        peer = np.arange((1 - h) * DH, (2 - h) * DH)
        perm = np.concatenate([own, peer])
        W_sel = np.concatenate([W_in[perm], W_in[DI + own]], 0)  # (2304, 768)
        maps.append({
            "x_in": np.ascontiguousarray(x[b]),
            "w_in_t": np.ascontiguousarray((W_sel * ln_g[None, :]).T).astype(NPBF16),
            "bias_in": np.ascontiguousarray(W_sel @ ln_b),
            "w_conv": np.ascontiguousarray(W_conv[perm, 0, :]),
            "b_conv": np.ascontiguousarray(b_conv[perm]),
            "w_x_t": np.ascontiguousarray(W_x[np.r_[DTR:DTR + 2 * N, 0:DTR]][:, perm].T).astype(NPBF16),
            "w_dt_t": np.ascontiguousarray(W_dt[own].T).astype(NPBF16),
            "b_dt": np.ascontiguousarray(b_dt[own]),
            "a_neg": np.ascontiguousarray(-np.exp(A_log[own])).astype(NPBF16),
            "d_par": np.ascontiguousarray(D_param[own]),
            "w_out_t": np.ascontiguousarray(W_out[:, own].T).astype(NPBF16),
        })
    return maps


_CACHED = {}


def kernel(**inputs) -> np.ndarray:
    if "nc" not in _CACHED:
        _CACHED["nc"] = build_program()
    nc = _CACHED["nc"]
    maps = make_in_maps(inputs)
    res = run_bass_kernel_spmd(nc, maps, core_ids=list(range(8)))
    out = np.stack([res.results[2 * b]["out"] for b in range(B)], 0)
    return out.astype(np.float32)


if __name__ == "__main__":
    rng = np.random.default_rng(0)
    ins = {
        "input_data": rng.standard_normal((B, L, D), np.float32),
        "ln_g": np.ones((D,), np.float32),
        "ln_b": np.zeros((D,), np.float32),
        "W_in": rng.standard_normal((2 * DI, D), np.float32) * 0.02,
        "W_conv": rng.standard_normal((DI, 1, DCONV), np.float32) * 0.02,
        "b_conv": np.zeros((DI,), np.float32),
        "W_x": rng.standard_normal((DTR + 2 * N, DI), np.float32) * 0.02,
        "W_dt": rng.standard_normal((DI, DTR), np.float32) * 0.02,
        "b_dt": rng.standard_normal((DI,), np.float32) * 0.1,
        "A_log": np.log(np.broadcast_to(np.arange(1, N + 1, dtype=np.float32), (DI, N))).copy(),
        "D_param": np.ones((DI,), np.float32),
        "W_out": rng.standard_normal((D, DI), np.float32) * 0.02,
    }
    out = kernel(**ins)
    print("kernel out", out.shape, out.dtype)

